# revision 1
# baseline (speedup 1.0000x reference)
"""Trainium2 Bass kernel for the BoundaryLoss problem.

Computes mean(ce * w) where
  ce = -log_softmax(inputs)[targets]           (weighted cross entropy)
  w  = exp(-EDT(boundary(targets)) / sigma)    (boundary-distance weights)

Sharding: data-parallel over batch, one image per NeuronCore (B=8, 8 cores).
Each core emits per-partition partial sums [sum(ce*w), sum(ce), max(d2)];
the host folds partitions/cores and resolves the per-image "no boundary"
case (max(d2) > 1e11  =>  w == 1  =>  use sum(ce)).

Per-core pipeline (one [19,256,256] image), VectorE-bound by the EDT:
  1. boundary: 3x3 morphological gradient via separable 3-point min/max in
     bf16 (vertical pass in PE-transposed layout, horizontal pass natural).
  2. per-row 1D distance g with tensor_tensor_scan (fwd + reversed bwd),
     exactly the reference recurrence c = min(c+1, boundary ? 0 : 1e6).
  3. exact 2D EDT d2[i,j] = min_k((i-k)^2 + g2[k,j]) as a brute-force
     min-plus in the transposed layout [w-partitions, i-free]: per k one
     4x-mode tensor_scalar add of a sliding bf16 (i-k)^2 window table
     (two parity copies keep the window 4B-aligned) with the per-partition
     f32 g2 column as scalar, then a wide pairwise tensor_tensor bf16 min
     tree (2x mode; min winners are small integers so bf16 is near-exact).
  4. w = exp(-sqrt(d2)/5) on ScalarE (sqrt/exp grouped by activation table
     set so loads hide under EDT work).
  5. ce = log(sum_c exp(x_c)) - x[target]: exp + per-class equality masks
     (relu(1-|t-c|) -> u8) on ScalarE, channel-sum as a bf16 add tree and
     the target gather as copy_predicated on VectorE; this VectorE work is
     slotted between the two EDT halves so the in-order DVE stream never
     stalls on the 4.75MB activations DMA.
  6. ce is PE-transposed mid-kernel so the tail is just exp -> mul ->
     reduce; all small constants arrive in one byte-packed DMA (per-DMA
     queue cost ~2us) and activations stream on the gpsimd DMA queue.
"""

import numpy as np
import ml_dtypes
from contextlib import ExitStack

import concourse.bacc as bacc
import concourse.tile as tile
from concourse import mybir
from concourse.bass_utils import run_bass_kernel_spmd

F32 = mybir.dt.float32
BF16 = mybir.dt.bfloat16
I32 = mybir.dt.int32
U8 = mybir.dt.uint8
Alu = mybir.AluOpType
Act = mybir.ActivationFunctionType
AX = mybir.AxisListType

B, C, H, W = 8, 19, 256, 256
N_CORES = 8
P = 128
HT = H // P  # 2 h-tiles (natural layout: h on partitions)
WT = W // P  # 2 w-tiles (transposed layout: w on partitions)
INF = 1.0e6
SIGMA = 5.0
KCHUNK = 64  # k's per EDT chunk (evens+odds wide tiles of 32*256 bf16)
CB_BYTES = 3584 + 4 * (C + 1)  # packed constant bundle bytes per partition


def _win(dwA, dwB, k):
    """bf16 sliding window AP for (i-k)^2 over i=0..255, 4B-aligned start."""
    off = 255 - k
    if off % 2 == 0:
        return dwA[:, off:off + 256]
    off = 254 - k
    return dwB[:, off:off + 256]


def build():
    nc = bacc.Bacc("TRN2", target_bir_lowering=False, debug=False)
    x_d = nc.dram_tensor("x", [C, H, W], F32, kind="ExternalInput").ap()
    t_d = nc.dram_tensor("t", [H, W], I32, kind="ExternalInput").ap()
    idnb_d = nc.dram_tensor("idnb", [P, P], BF16, kind="ExternalInput").ap()
    cb_d = nc.dram_tensor("cb", [P, CB_BYTES], U8, kind="ExternalInput").ap()
    out_d = nc.dram_tensor("out", [P, 4], F32, kind="ExternalOutput").ap()

    with tile.TileContext(nc) as tc, ExitStack() as ctx:
        cp = ctx.enter_context(tc.tile_pool(name="consts", bufs=1))
        wp = ctx.enter_context(tc.tile_pool(name="work", bufs=1))
        sp = ctx.enter_context(tc.tile_pool(name="scratch", bufs=3))
        ep = ctx.enter_context(tc.tile_pool(name="edt", bufs=1))
        pp = ctx.enter_context(tc.tile_pool(name="psum", bufs=2, space="PSUM"))

        # ---- inputs: one target DMA (combined layout) + one bundled
        # constant DMA (per-dma_start queue cost is ~2us, so batching the
        # small constants is a real latency win) ----
        # combined layout: partition p <-> h = a*128+p, free = (a, w);
        # slice [:, a*256:(a+1)*256] is exactly natural h-tile a
        t2_i = wp.tile([P, 2 * W], I32, tag="t2i")
        nc.sync.dma_start(t2_i[:].rearrange("p (a w) -> p a w", a=2),
                          t_d.rearrange("(a p) w -> p a w", a=2))
        idnb = cp.tile([P, P], BF16, tag="idnb")
        nc.sync.dma_start(idnb[:], idnb_d[:])
        cb = cp.tile([P, CB_BYTES], U8, tag="cb")
        nc.sync.dma_start(cb[:], cb_d[:])
        dwA = cb[:, 0:1024].bitcast(BF16)
        dwB = cb[:, 1024:2048].bitcast(BF16)
        idn = cb[:, 2048:2560].bitcast(F32)
        ones = cb[:, 2560:3584].bitcast(F32)
        cneg = cb[:, 3584:3584 + 4 * (C + 1)].bitcast(F32)

        t2_f = wp.tile([P, 2 * W], F32, tag="t2f")
        nc.vector.tensor_copy(t2_f[:], t2_i[:])
        t2_b = wp.tile([P, 2 * W], BF16, tag="t2b")
        nc.vector.tensor_copy(t2_b[:], t2_f[:])
        tb = [t2_b[:, ht * 256:(ht + 1) * 256] for ht in range(HT)]

        # X layout: [p, (c, a, w)] - on the gpsimd DMA path, off the
        # sync-engine queue that carries the small latency-critical loads
        X = wp.tile([P, C * 2 * W], F32, tag="X")
        nc.gpsimd.dma_start(
            X[:].rearrange("p (c a w) -> p c a w", c=C, a=2),
            x_d.rearrange("c (a p) w -> p c a w", a=2))

        # ---- transpose helper: 2 natural [P,256] -> 2 transposed [P,256] ----
        def transpose_256(src_tiles, dst_tag, dst_dt=F32, src_bf=False):
            ident = idnb if src_bf else idn
            outs = []
            for o in range(2):
                ps = pp.tile([P, 256], BF16 if src_bf else F32,
                             tag="tpb" if src_bf else "tp")
                for s_ in range(2):
                    nc.tensor.transpose(
                        ps[:, s_ * P:(s_ + 1) * P],
                        src_tiles[s_][:, o * P:(o + 1) * P],
                        ident[:],
                    )
                dst = wp.tile([P, 256], dst_dt, tag=f"{dst_tag}{o}")
                nc.scalar.copy(dst[:], ps[:])
                outs.append(dst)
            return outs

        # ---- boundary in bf16: fused transpose->padded tiles ----
        def transpose_pad(src_tiles):
            """2 natural bf16 [P,256] -> 2 transposed edge-padded [P,258]."""
            pads = []
            for o in range(2):
                ps = pp.tile([P, 256], BF16, tag="tpb")
                for s_ in range(2):
                    nc.tensor.transpose(
                        ps[:, s_ * P:(s_ + 1) * P],
                        src_tiles[s_][:, o * P:(o + 1) * P],
                        idnb[:],
                    )
                pad = sp.tile([P, 258], BF16, tag="pad3")
                nc.scalar.copy(pad[:, 1:257], ps[:])
                nc.scalar.copy(pad[:, 0:1], ps[:, 0:1])
                nc.scalar.copy(pad[:, 257:258], ps[:, 255:256])
                pads.append(pad)
            return pads

        def filt3p(pads, tag, op):
            outs = []
            for i, pad in enumerate(pads):
                r = wp.tile([P, 256], BF16, tag=f"{tag}{i}")
                nc.vector.tensor_tensor(r[:], pad[:, 0:256], pad[:, 1:257], op)
                nc.vector.tensor_tensor(r[:], r[:], pad[:, 2:258], op)
                outs.append(r)
            return outs

        padT = transpose_pad(tb)
        vmaxT = filt3p(padT, "vmaxT", Alu.max)
        vminT = filt3p(padT, "vminT", Alu.min)
        hmax = filt3p(transpose_pad(vmaxT), "hmax", Alu.max)
        hmin = filt3p(transpose_pad(vminT), "hmin", Alu.min)

        ind = []
        for ht in range(HT):
            d = sp.tile([P, 256], BF16, tag="bdiff")
            nc.vector.tensor_tensor(d[:], hmax[ht][:], hmin[ht][:], Alu.subtract)
            # ind = (diff == 0) * INF : INF where NOT boundary, 0 on boundary
            iv = wp.tile([P, 256], F32, tag=f"ind{ht}")
            nc.vector.tensor_scalar(iv[:], d[:], 0.0, INF, Alu.is_equal, Alu.mult)
            ind.append(iv)

        # ---- per-row distance (scan fwd/bwd) and g^2 ----
        g2 = []
        for ht in range(HT):
            fwd = sp.tile([P, 256], F32, tag="fwd")
            nc.vector.tensor_tensor_scan(fwd[:], ones[:], ind[ht][:], INF,
                                         Alu.add, Alu.min)
            bwr = sp.tile([P, 256], F32, tag="bwr")
            nc.vector.tensor_tensor_scan(bwr[:], ones[:], ind[ht][:, ::-1], INF,
                                         Alu.add, Alu.min)
            g = sp.tile([P, 256], F32, tag="g")
            nc.vector.tensor_tensor(g[:], fwd[:], bwr[:, ::-1], Alu.min)
            g2t = wp.tile([P, 256], F32, tag=f"g2{ht}")
            nc.vector.tensor_tensor(g2t[:], g[:], g[:], Alu.mult)
            g2.append(g2t)

        g2T = transpose_256(g2, "g2T", dst_dt=F32)

        # ---- CE: ScalarE work emitted early (exp + class masks) ----
        S = 2 * W  # 512 pixels per partition
        ex = wp.tile([P, C * S], BF16, tag="Ex")
        nc.scalar.activation(ex[:], X[:], Act.Exp)
        masks = []
        for c in range(1, C):
            ab = sp.tile([P, S], F32, tag="mab")
            nc.scalar.activation(ab[:], t2_f[:], Act.Abs, bias=cneg[:, c:c + 1])
            m = wp.tile([P, S], U8, tag=f"mask{c}")
            nc.scalar.activation(m[:], ab[:], Act.Relu, bias=ones[:, 0:1],
                                 scale=-1.0)
            masks.append(m)

        # ---- EDT min-plus: d2T[j, i] = min_k ((i-k)^2 + g2T[j, k]) ----
        # chunk sizes chosen so the wide pairwise min tree amortizes the
        # per-op overhead; 96+96+64 covers k=0..255
        chunk_plan = [(0, 64), (64, 64), (128, 64), (192, 64)]
        d2T = []
        for wt in range(WT):
            cres = sp.tile([P, len(chunk_plan) * 256], BF16, tag="cres")
            for ci, (c0, clen) in enumerate(chunk_plan):
                npair = clen // 2
                ev = ep.tile([P, npair * 256], BF16, tag="ev")
                od = ep.tile([P, npair * 256], BF16, tag="od")
                for m_ in range(npair):
                    k0 = c0 + 2 * m_
                    nc.vector.tensor_scalar(
                        ev[:, m_ * 256:(m_ + 1) * 256], _win(dwA, dwB, k0),
                        g2T[wt][:, k0:k0 + 1], None, Alu.add)
                    nc.vector.tensor_scalar(
                        od[:, m_ * 256:(m_ + 1) * 256], _win(dwA, dwB, k0 + 1),
                        g2T[wt][:, k0 + 1:k0 + 2], None, Alu.add)
                nc.vector.tensor_tensor(ev[:], ev[:], od[:], Alu.min)
                nblk = npair  # 256-wide blocks remaining in ev
                while nblk > 2:
                    if nblk % 2 == 1:
                        # fold the odd tail block into block 0
                        nc.vector.tensor_tensor(
                            ev[:, 0:256], ev[:, 0:256],
                            ev[:, (nblk - 1) * 256:nblk * 256], Alu.min)
                        nblk -= 1
                    half = nblk // 2 * 256
                    nc.vector.tensor_tensor(ev[:, 0:half], ev[:, 0:half],
                                            ev[:, half:2 * half], Alu.min)
                    nblk //= 2
                nc.vector.tensor_tensor(cres[:, ci * 256:(ci + 1) * 256],
                                        ev[:, 0:256], ev[:, 256:512], Alu.min)
            acc = wp.tile([P, 256], BF16, tag=f"d2T{wt}")
            acc_inst = nc.vector.tensor_tensor(
                acc[:], cres[:, 0:256], cres[:, 256:512], Alu.min)
            for ci in range(2, len(chunk_plan)):
                acc_inst = nc.vector.tensor_tensor(
                    acc[:], acc[:], cres[:, ci * 256:(ci + 1) * 256], Alu.min)
            d2T.append(acc)
            if wt == 0:
                # ---- CE DVE work, slotted between the two EDT halves so the
                # in-order DVE stream never stalls on the X DMA ----
                ce0_inst = nc.vector.tensor_tensor(ex[:, 0:8 * S], ex[:, 0:8 * S],
                                                   ex[:, 8 * S:16 * S], Alu.add)
                tile.add_dep_helper(ce0_inst.ins, acc_inst.ins, False,
                                    "keep CE after EDT half 0")
                nc.vector.tensor_tensor(ex[:, 0:4 * S], ex[:, 0:4 * S],
                                        ex[:, 4 * S:8 * S], Alu.add)
                nc.vector.tensor_tensor(ex[:, 0:2 * S], ex[:, 0:2 * S],
                                        ex[:, 2 * S:4 * S], Alu.add)
                nc.vector.tensor_tensor(ex[:, 0:S], ex[:, 0:S], ex[:, S:2 * S],
                                        Alu.add)
                tail = sp.tile([P, S], BF16, tag="tail")
                nc.vector.tensor_tensor(tail[:], ex[:, 16 * S:17 * S],
                                        ex[:, 17 * S:18 * S], Alu.add)
                nc.vector.tensor_tensor(tail[:], tail[:], ex[:, 18 * S:19 * S],
                                        Alu.add)
                esum = sp.tile([P, S], F32, tag="esum")
                nc.vector.tensor_tensor(esum[:], ex[:, 0:S], tail[:], Alu.add)
                lse = sp.tile([P, S], F32, tag="lse")
                nc.scalar.activation(lse[:], esum[:], Act.Ln)
                xt = sp.tile([P, S], F32, tag="xt")
                xt_inst = nc.vector.tensor_copy(xt[:], X[:, 0:S])
                tile.add_dep_helper(xt_inst.ins, acc_inst.ins, False,
                                    "keep gather after EDT half 0")
                for c in range(1, C):
                    nc.vector.copy_predicated(xt[:], masks[c - 1][:],
                                              X[:, c * S:(c + 1) * S])
                ce = wp.tile([P, S], F32, tag="ce")
                nc.vector.tensor_tensor(ce[:], lse[:], xt[:], Alu.subtract)
                ceT = transpose_256([ce[:, 0:256], ce[:, 256:512]], "ceT")

        # ---- w = exp(-sqrt(d2)/sigma) in transposed layout; the
        # no-boundary-image case is resolved host-side via max(d2) ----
        wTs = []
        for wt in range(WT):
            w_t = wp.tile([P, 256], F32, tag=f"wT{wt}")
            nc.scalar.activation(w_t[:], d2T[wt][:], Act.Sqrt)
            wTs.append(w_t)
        # ---- outputs: per-partition [sum(ce*w), sum(ce), max(d2)] ----
        # products in the transposed layout (ce was transposed mid-kernel),
        # so the tail is just exp -> mul -> reduce
        ot = wp.tile([P, 4], F32, tag="ot")
        nc.vector.tensor_reduce(ot[:, 1:2], ce[:], AX.X, Alu.add)
        dm = wp.tile([P, HT], F32, tag="dm")
        nc.vector.tensor_reduce(dm[:, 0:1], d2T[0][:], AX.X, Alu.max)
        sw = wp.tile([P, WT], F32, tag="s")
        for wt in range(WT):
            nc.scalar.activation(wTs[wt][:], wTs[wt][:], Act.Exp,
                                 scale=-1.0 / SIGMA)
            prod = sp.tile([P, 256], F32, tag="prod")
            nc.vector.tensor_tensor(prod[:], ceT[wt][:], wTs[wt][:], Alu.mult)
            nc.vector.tensor_reduce(sw[:, wt:wt + 1], prod[:], AX.X, Alu.add)
        nc.vector.tensor_reduce(dm[:, 1:2], d2T[1][:], AX.X, Alu.max)
        nc.vector.tensor_reduce(ot[:, 0:1], sw[:], AX.X, Alu.add)
        nc.vector.tensor_reduce(ot[:, 2:3], dm[:], AX.X, Alu.max)
        nc.vector.tensor_copy(ot[:, 3:4], ot[:, 2:3])
        nc.sync.dma_start(out_d[:], ot[:])

    nc.compile()
    return nc


def make_consts():
    cvals = (np.arange(512, dtype=np.float64) - 255.0) ** 2
    dwA = np.broadcast_to(cvals, (P, 512)).astype(ml_dtypes.bfloat16)
    cvals2 = (np.arange(512, dtype=np.float64) - 254.0) ** 2
    dwB = np.broadcast_to(cvals2, (P, 512)).astype(ml_dtypes.bfloat16)
    idn = np.eye(P, dtype=np.float32)
    idnb = np.eye(P, dtype=np.float32).astype(ml_dtypes.bfloat16)
    ones = np.ones((P, 256), np.float32)
    cneg = np.broadcast_to(-np.arange(C + 1, dtype=np.float32), (P, C + 1))
    cb = np.concatenate([
        np.ascontiguousarray(dwA).view(np.uint8),
        np.ascontiguousarray(dwB).view(np.uint8),
        idn.view(np.uint8),
        ones.view(np.uint8),
        np.ascontiguousarray(cneg).astype(np.float32).view(np.uint8),
    ], axis=1)
    assert cb.shape == (P, CB_BYTES), cb.shape
    return {"cb": np.ascontiguousarray(cb), "idnb": np.ascontiguousarray(idnb)}


_NC = None


def _get_nc():
    global _NC
    if _NC is None:
        _NC = build()
    return _NC


def kernel(**inputs):
    x = np.asarray(inputs["inputs"], dtype=np.float32)
    t = np.asarray(inputs["targets"])
    if t.dtype != np.int32:
        t = t.astype(np.int32)
    assert x.shape == (B, C, H, W) and t.shape == (B, H, W)
    nc = _get_nc()
    consts = make_consts()
    in_maps = [dict(x=x[b], t=t[b], **consts) for b in range(B)]
    res = run_bass_kernel_spmd(nc, in_maps, core_ids=list(range(N_CORES)))
    total = 0.0
    for b in range(B):
        o = res.results[b]["out"]  # [128, 4]: sum(ce*w), sum(ce), max(d2), pad
        has_boundary = float(o[:, 2].max()) <= 1.0e11
        total += float(o[:, 0].sum()) if has_boundary else float(o[:, 1].sum())
    return np.float32(total / (B * H * W))



# revision 5
# speedup vs baseline: 1.0072x; 1.0072x over previous
"""Trainium2 Bass kernel for the BoundaryLoss problem.

Computes mean(ce * w) where
  ce = -log_softmax(inputs)[targets]           (weighted cross entropy)
  w  = exp(-EDT(boundary(targets)) / sigma)    (boundary-distance weights)

Sharding: data-parallel over batch, one image per NeuronCore (B=8, 8 cores).
Each core emits per-partition partial sums [sum(ce*w), sum(ce), max(d2)];
the host folds partitions/cores and resolves the per-image "no boundary"
case (max(d2) > 1e11  =>  w == 1  =>  use sum(ce)).

Dispatch-latency design (the end-to-end call is transfer/dispatch bound,
not compute bound -- the on-chip kernel is ~0.2 ms while a PJRT dispatch
through the tunnel costs hundreds of ms):
  * ONE input tensor per core: bf16 [20, 256, 256] = logits channels 0..18
    plus the targets as an exact bf16 channel 19 (values 0..18 are exact
    in bf16). 2.62 MB/core vs 5.5 MB/core across 4 tensors for the f32
    layout; host-side f32->bf16 RNE conversion costs ~27 ms and changes
    the final loss by ~1e-6 relative.
  * every constant (window tables, transpose identities, ones, class
    offsets) is generated on-chip with gpsimd iota/memset instead of DMAs.
  * the jitted shard_map dispatch is built once and cached at module
    scope; run_bass_kernel_spmd re-traces jax on every call (fresh _body
    closure), which costs ~0.15 s/call on top of the transfers.

Per-core pipeline (one [19,256,256] image), VectorE-bound by the EDT:
  1. boundary: 3x3 morphological gradient via separable 3-point min/max in
     bf16 (vertical pass in PE-transposed layout, horizontal pass natural).
  2. per-row 1D distance g with tensor_tensor_scan (fwd + reversed bwd),
     exactly the reference recurrence c = min(c+1, boundary ? 0 : 1e6).
  3. exact 2D EDT d2[i,j] = min_k((i-k)^2 + g2[k,j]) as a brute-force
     min-plus in the transposed layout [w-partitions, i-free]: per k one
     4x-mode tensor_scalar add of a sliding bf16 (i-k)^2 window table
     (two parity copies keep the window 4B-aligned) with the per-partition
     f32 g2 column as scalar, then a wide pairwise tensor_tensor bf16 min
     tree (2x mode; min winners are small integers so bf16 is near-exact).
  4. w = exp(-sqrt(d2)/5) on ScalarE (sqrt/exp grouped by activation table
     set so loads hide under EDT work).
  5. ce = log(sum_c exp(x_c)) - x[target]: exp + per-class equality masks
     (relu(1-|t-c|) -> u8) on ScalarE, channel-sum as a bf16 add tree and
     the target gather as copy_predicated on VectorE; this VectorE work is
     slotted between the two EDT halves so the in-order DVE stream never
     stalls on the logits DMA.
  6. ce is PE-transposed mid-kernel so the tail is just exp -> mul ->
     reduce; the targets channel lands first on the sync DMA queue while
     the 19 logit channels stream on the gpsimd DMA queue.
"""

import numpy as np
import ml_dtypes
from contextlib import ExitStack

import concourse.bacc as bacc
import concourse.tile as tile
from concourse import mybir

F32 = mybir.dt.float32
BF16 = mybir.dt.bfloat16
I32 = mybir.dt.int32
U8 = mybir.dt.uint8
Alu = mybir.AluOpType
Act = mybir.ActivationFunctionType
AX = mybir.AxisListType

B, C, H, W = 8, 19, 256, 256
CT = C + 1  # shipped channels: 19 logits + targets
N_CORES = 8
P = 128
HT = H // P  # 2 h-tiles (natural layout: h on partitions)
WT = W // P  # 2 w-tiles (transposed layout: w on partitions)
INF = 1.0e6
SIGMA = 5.0


def _win(dwA, dwB, k):
    """bf16 sliding window AP for (i-k)^2 over i=0..255, 4B-aligned start."""
    off = 255 - k
    if off % 2 == 0:
        return dwA[:, off:off + 256]
    off = 254 - k
    return dwB[:, off:off + 256]


def build():
    nc = bacc.Bacc("TRN2", target_bir_lowering=False, debug=False)
    x_d = nc.dram_tensor("x", [CT, H, W], BF16, kind="ExternalInput").ap()
    out_d = nc.dram_tensor("out", [P, 4], F32, kind="ExternalOutput").ap()

    with tile.TileContext(nc) as tc, ExitStack() as ctx:
        cp = ctx.enter_context(tc.tile_pool(name="consts", bufs=1))
        wp = ctx.enter_context(tc.tile_pool(name="work", bufs=1))
        sp = ctx.enter_context(tc.tile_pool(name="scratch", bufs=3))
        ep = ctx.enter_context(tc.tile_pool(name="edt", bufs=1))
        pp = ctx.enter_context(tc.tile_pool(name="psum", bufs=2, space="PSUM"))

        # ---- constants, generated on-chip (gpsimd, emitted FIRST so the
        # Pool engine produces the PE-transpose identity by ~1.2us; the
        # DMA triggers below cost ~0.5/7.5us on their issuing engine) ----
        rmp = cp.tile([P, P], I32, tag="rmp")  # free_idx - partition_idx
        nc.gpsimd.iota(rmp[:], [[1, P]], channel_multiplier=-1)
        idn = cp.tile([P, P], F32, tag="idn")  # eye(128) for PE transpose
        nc.gpsimd.tensor_scalar(idn[:], rmp[:], 0, None, Alu.is_equal)
        idnb = cp.tile([P, P], BF16, tag="idnb")
        nc.gpsimd.tensor_copy(idnb[:], idn[:])
        it512 = cp.tile([P, 512], I32, tag="it512")
        nc.gpsimd.iota(it512[:], [[1, 512]], channel_multiplier=0)
        cneg = cp.tile([P, C], F32, tag="cneg")  # -c for the class masks
        nc.gpsimd.tensor_scalar(cneg[:], it512[:, 0:C], -1.0, None, Alu.mult)
        ones = cp.tile([P, 256], F32, tag="ones")
        nc.gpsimd.memset(ones[:], 1.0)
        f512 = cp.tile([P, 512], F32, tag="f512")
        nc.gpsimd.tensor_copy(f512[:], it512[:])
        dtmp = cp.tile([P, 512], F32, tag="dtmp")
        dwA = cp.tile([P, 512], BF16, tag="dwA")  # (i-255)^2, i=0..511
        nc.gpsimd.tensor_scalar(dtmp[:], f512[:], 255.0, None, Alu.subtract)
        nc.gpsimd.tensor_tensor(dwA[:], dtmp[:], dtmp[:], Alu.mult)
        dwB = cp.tile([P, 512], BF16, tag="dwB")  # (i-254)^2
        nc.gpsimd.tensor_scalar(dtmp[:], f512[:], 254.0, None, Alu.subtract)
        nc.gpsimd.tensor_tensor(dwB[:], dtmp[:], dtmp[:], Alu.mult)

        # ---- inputs: targets channel first on the sync queue (the whole
        # boundary/EDT pipeline hangs off it), then the logits on the same
        # queue (SP is otherwise idle; its 7.5us trigger cost is hidden).
        # combined layout: partition p <-> h = a*128+p, free = (a, w);
        # slice [:, a*256:(a+1)*256] is exactly natural h-tile a ----
        t2_b = wp.tile([P, 2 * W], BF16, tag="t2b")
        nc.sync.dma_start(
            t2_b[:].rearrange("p (c a w) -> p c a w", c=1, a=2),
            x_d[C:CT].rearrange("c (a p) w -> p c a w", a=2))
        X = wp.tile([P, C * 2 * W], BF16, tag="X")
        nc.sync.dma_start(
            X[:].rearrange("p (c a w) -> p c a w", c=C, a=2),
            x_d[0:C].rearrange("c (a p) w -> p c a w", a=2))

        t2_f = wp.tile([P, 2 * W], F32, tag="t2f")
        nc.scalar.copy(t2_f[:], t2_b[:])
        tb = [t2_b[:, ht * 256:(ht + 1) * 256] for ht in range(HT)]

        # ---- transpose helper: 2 natural [P,256] -> 2 transposed [P,256] ----
        act_copies = []  # scalar-engine copy insts, for ordering pins

        def transpose_256(src_tiles, dst_tag, dst_dt=F32, src_bf=False):
            ident = idnb if src_bf else idn
            outs = []
            for o in range(2):
                ps = pp.tile([P, 256], BF16 if src_bf else F32,
                             tag="tpb" if src_bf else "tp")
                for s_ in range(2):
                    nc.tensor.transpose(
                        ps[:, s_ * P:(s_ + 1) * P],
                        src_tiles[s_][:, o * P:(o + 1) * P],
                        ident[:],
                    )
                dst = wp.tile([P, 256], dst_dt, tag=f"{dst_tag}{o}")
                act_copies.append(nc.scalar.copy(dst[:], ps[:]))
                outs.append(dst)
            return outs

        # ---- boundary in bf16: fused transpose->padded tiles ----
        def transpose_pad(src_tiles):
            """2 natural bf16 [P,256] -> 2 transposed edge-padded [P,258]."""
            pads = []
            for o in range(2):
                ps = pp.tile([P, 256], BF16, tag="tpb")
                for s_ in range(2):
                    nc.tensor.transpose(
                        ps[:, s_ * P:(s_ + 1) * P],
                        src_tiles[s_][:, o * P:(o + 1) * P],
                        idnb[:],
                    )
                pad = sp.tile([P, 258], BF16, tag="pad3")
                nc.scalar.copy(pad[:, 1:257], ps[:])
                nc.scalar.copy(pad[:, 0:1], ps[:, 0:1])
                nc.scalar.copy(pad[:, 257:258], ps[:, 255:256])
                pads.append(pad)
            return pads

        def filt3p(pads, tag, op):
            outs = []
            for i, pad in enumerate(pads):
                r = wp.tile([P, 256], BF16, tag=f"{tag}{i}")
                nc.vector.tensor_tensor(r[:], pad[:, 0:256], pad[:, 1:257], op)
                nc.vector.tensor_tensor(r[:], r[:], pad[:, 2:258], op)
                outs.append(r)
            return outs

        padT = transpose_pad(tb)
        vmaxT = filt3p(padT, "vmaxT", Alu.max)
        vminT = filt3p(padT, "vminT", Alu.min)
        hmax = filt3p(transpose_pad(vmaxT), "hmax", Alu.max)
        hmin = filt3p(transpose_pad(vminT), "hmin", Alu.min)

        ind = []
        for ht in range(HT):
            d = sp.tile([P, 256], BF16, tag="bdiff")
            nc.vector.tensor_tensor(d[:], hmax[ht][:], hmin[ht][:], Alu.subtract)
            # ind = (diff == 0) * INF : INF where NOT boundary, 0 on boundary
            iv = wp.tile([P, 256], F32, tag=f"ind{ht}")
            nc.vector.tensor_scalar(iv[:], d[:], 0.0, INF, Alu.is_equal, Alu.mult)
            ind.append(iv)

        # ---- per-row distance (scan fwd/bwd) and g^2 ----
        g2 = []
        for ht in range(HT):
            fwd = sp.tile([P, 256], F32, tag="fwd")
            nc.vector.tensor_tensor_scan(fwd[:], ones[:], ind[ht][:], INF,
                                         Alu.add, Alu.min)
            bwr = sp.tile([P, 256], F32, tag="bwr")
            nc.vector.tensor_tensor_scan(bwr[:], ones[:], ind[ht][:, ::-1], INF,
                                         Alu.add, Alu.min)
            g = sp.tile([P, 256], F32, tag="g")
            nc.vector.tensor_tensor(g[:], fwd[:], bwr[:, ::-1], Alu.min)
            g2t = wp.tile([P, 256], F32, tag=f"g2{ht}")
            nc.vector.tensor_tensor(g2t[:], g[:], g[:], Alu.mult)
            g2.append(g2t)

        g2T = transpose_256(g2, "g2T", dst_dt=F32)

        # ---- CE: ScalarE work emitted early (exp + class masks) ----
        S = 2 * W  # 512 pixels per partition
        ex = wp.tile([P, C * S], BF16, tag="Ex")
        ex_inst = nc.scalar.activation(ex[:], X[:], Act.Exp)
        # the 8.3us exp must not jump the Act queue ahead of the boundary
        # pipeline's pad/transpose copies (it stalls DVE for ~8us otherwise)
        tile.add_dep_helper(ex_inst.ins, act_copies[-1].ins, False,
                            "exp after g2T copies")
        masks = []
        for c in range(1, C):
            ab = sp.tile([P, S], F32, tag="mab")
            nc.scalar.activation(ab[:], t2_f[:], Act.Abs, bias=cneg[:, c:c + 1])
            m = wp.tile([P, S], U8, tag=f"mask{c}")
            nc.scalar.activation(m[:], ab[:], Act.Relu, bias=ones[:, 0:1],
                                 scale=-1.0)
            masks.append(m)

        # ---- EDT min-plus: d2T[j, i] = min_k ((i-k)^2 + g2T[j, k]) ----
        chunk_plan = [(0, 64), (64, 64), (128, 64), (192, 64)]
        d2T = []
        for wt in range(WT):
            cres = sp.tile([P, len(chunk_plan) * 256], BF16, tag="cres")
            for ci, (c0, clen) in enumerate(chunk_plan):
                npair = clen // 2
                ev = ep.tile([P, npair * 256], BF16, tag="ev")
                od = ep.tile([P, npair * 256], BF16, tag="od")
                for m_ in range(npair):
                    k0 = c0 + 2 * m_
                    nc.vector.tensor_scalar(
                        ev[:, m_ * 256:(m_ + 1) * 256], _win(dwA, dwB, k0),
                        g2T[wt][:, k0:k0 + 1], None, Alu.add)
                    nc.vector.tensor_scalar(
                        od[:, m_ * 256:(m_ + 1) * 256], _win(dwA, dwB, k0 + 1),
                        g2T[wt][:, k0 + 1:k0 + 2], None, Alu.add)
                nc.vector.tensor_tensor(ev[:], ev[:], od[:], Alu.min)
                nblk = npair  # 256-wide blocks remaining in ev
                while nblk > 2:
                    if nblk % 2 == 1:
                        # fold the odd tail block into block 0
                        nc.vector.tensor_tensor(
                            ev[:, 0:256], ev[:, 0:256],
                            ev[:, (nblk - 1) * 256:nblk * 256], Alu.min)
                        nblk -= 1
                    half = nblk // 2 * 256
                    nc.vector.tensor_tensor(ev[:, 0:half], ev[:, 0:half],
                                            ev[:, half:2 * half], Alu.min)
                    nblk //= 2
                nc.vector.tensor_tensor(cres[:, ci * 256:(ci + 1) * 256],
                                        ev[:, 0:256], ev[:, 256:512], Alu.min)
            acc = wp.tile([P, 256], BF16, tag=f"d2T{wt}")
            acc_inst = nc.vector.tensor_tensor(
                acc[:], cres[:, 0:256], cres[:, 256:512], Alu.min)
            for ci in range(2, len(chunk_plan)):
                acc_inst = nc.vector.tensor_tensor(
                    acc[:], acc[:], cres[:, ci * 256:(ci + 1) * 256], Alu.min)
            d2T.append(acc)
            if wt == 0:
                # ---- CE DVE work, slotted between the two EDT halves so the
                # in-order DVE stream never stalls on the X DMA ----
                ce0_inst = nc.vector.tensor_tensor(ex[:, 0:8 * S], ex[:, 0:8 * S],
                                                   ex[:, 8 * S:16 * S], Alu.add)
                tile.add_dep_helper(ce0_inst.ins, acc_inst.ins, False,
                                    "keep CE after EDT half 0")
                nc.vector.tensor_tensor(ex[:, 0:4 * S], ex[:, 0:4 * S],
                                        ex[:, 4 * S:8 * S], Alu.add)
                nc.vector.tensor_tensor(ex[:, 0:2 * S], ex[:, 0:2 * S],
                                        ex[:, 2 * S:4 * S], Alu.add)
                nc.vector.tensor_tensor(ex[:, 0:S], ex[:, 0:S], ex[:, S:2 * S],
                                        Alu.add)
                tail = sp.tile([P, S], BF16, tag="tail")
                nc.vector.tensor_tensor(tail[:], ex[:, 16 * S:17 * S],
                                        ex[:, 17 * S:18 * S], Alu.add)
                nc.vector.tensor_tensor(tail[:], tail[:], ex[:, 18 * S:19 * S],
                                        Alu.add)
                esum = sp.tile([P, S], F32, tag="esum")
                nc.vector.tensor_tensor(esum[:], ex[:, 0:S], tail[:], Alu.add)
                lse = sp.tile([P, S], F32, tag="lse")
                nc.scalar.activation(lse[:], esum[:], Act.Ln)
                xt = sp.tile([P, S], BF16, tag="xt")
                xt_inst = nc.vector.tensor_copy(xt[:], X[:, 0:S])
                tile.add_dep_helper(xt_inst.ins, acc_inst.ins, False,
                                    "keep gather after EDT half 0")
                for c in range(1, C):
                    nc.vector.copy_predicated(xt[:], masks[c - 1][:],
                                              X[:, c * S:(c + 1) * S])
                ce = wp.tile([P, S], F32, tag="ce")
                nc.vector.tensor_tensor(ce[:], lse[:], xt[:], Alu.subtract)
                ceT = transpose_256([ce[:, 0:256], ce[:, 256:512]], "ceT")

        # ---- w = exp(-sqrt(d2)/sigma) in transposed layout; the
        # no-boundary-image case is resolved host-side via max(d2) ----
        wTs = []
        for wt in range(WT):
            w_t = wp.tile([P, 256], F32, tag=f"wT{wt}")
            nc.scalar.activation(w_t[:], d2T[wt][:], Act.Sqrt)
            wTs.append(w_t)
        # ---- outputs: per-partition [sum(ce*w), sum(ce), max(d2)] ----
        ot = wp.tile([P, 4], F32, tag="ot")
        nc.vector.tensor_reduce(ot[:, 1:2], ce[:], AX.X, Alu.add)
        dm = wp.tile([P, HT], F32, tag="dm")
        nc.vector.tensor_reduce(dm[:, 0:1], d2T[0][:], AX.X, Alu.max)
        sw = wp.tile([P, WT], F32, tag="s")
        for wt in range(WT):
            nc.scalar.activation(wTs[wt][:], wTs[wt][:], Act.Exp,
                                 scale=-1.0 / SIGMA)
            prod = sp.tile([P, 256], F32, tag="prod")
            nc.vector.tensor_tensor(prod[:], ceT[wt][:], wTs[wt][:], Alu.mult)
            nc.vector.tensor_reduce(sw[:, wt:wt + 1], prod[:], AX.X, Alu.add)
        nc.vector.tensor_reduce(dm[:, 1:2], d2T[1][:], AX.X, Alu.max)
        nc.vector.tensor_reduce(ot[:, 0:1], sw[:], AX.X, Alu.add)
        nc.vector.tensor_reduce(ot[:, 2:3], dm[:], AX.X, Alu.max)
        nc.vector.tensor_copy(ot[:, 3:4], ot[:, 2:3])
        nc.sync.dma_start(out_d[:], ot[:])

    nc.compile()
    return nc


_LUT_BF16 = np.arange(C).astype(ml_dtypes.bfloat16).view(np.uint16)

_DISPATCH = None
_FALLBACK = None


def _get_dispatch():
    """Build nc + a cached jitted shard_map dispatch (once per process)."""
    global _DISPATCH
    if _DISPATCH is None:
        import jax
        import concourse.bass2jax as b2j

        nc = build()
        b2j.install_neuronx_cc_hook()
        if getattr(nc, "partition_id_tensor", None) is not None:
            raise RuntimeError("unexpected partition_id tensor")
        out_aval = jax.core.ShapedArray((P, 4), np.float32)

        def _body(xin, zout):
            outs = b2j._bass_exec_p.bind(
                xin, zout,
                out_avals=(out_aval,),
                in_names=("x", "out"),
                out_names=("out",),
                lowering_input_output_aliases=(),
                sim_require_finite=True,
                sim_require_nnan=True,
                nc=nc,
            )
            return tuple(outs)

        devices = jax.devices()[:N_CORES]
        assert len(devices) == N_CORES
        mesh = b2j.Mesh(np.asarray(devices), ("core",))
        fn = jax.jit(
            b2j.shard_map(_body, mesh=mesh,
                          in_specs=(b2j.PartitionSpec("core"),) * 2,
                          out_specs=(b2j.PartitionSpec("core"),),
                          check_rep=False),
            donate_argnums=(1,), keep_unused=True)
        _DISPATCH = (fn, nc)
    return _DISPATCH


def _pack_inputs(x, t):
    """f32 logits + int targets -> one bf16 [B*20, H, W] array (RNE)."""
    ship = np.empty((B, CT, H, W), np.uint16)
    ship[:, 0:C] = np.asarray(x, np.float32).astype(ml_dtypes.bfloat16).view(
        np.uint16)
    ship[:, C] = _LUT_BF16[np.asarray(t)]
    return ship.reshape(B * CT, H, W).view(ml_dtypes.bfloat16)


def _fold(o):
    """[B, P, 4] per-partition partials -> scalar loss."""
    total = 0.0
    for b in range(B):
        has_boundary = float(o[b, :, 2].max()) <= 1.0e11
        total += float(o[b, :, 0].sum()) if has_boundary else float(o[b, :, 1].sum())
    return np.float32(total / (B * H * W))


def kernel(**inputs):
    global _FALLBACK
    x = np.asarray(inputs["inputs"])
    t = np.asarray(inputs["targets"])
    assert x.shape == (B, C, H, W) and t.shape == (B, H, W)
    xg = _pack_inputs(x, t)
    if not _FALLBACK:
        try:
            fn, _ = _get_dispatch()
            zout = np.zeros((B * P, 4), np.float32)
            o = np.asarray(fn(xg, zout)[0]).reshape(B, P, 4)
            return _fold(o)
        except Exception:
            _FALLBACK = True
    from concourse.bass_utils import run_bass_kernel_spmd
    nc = _get_nc()
    in_maps = [{"x": np.asarray(xg.reshape(B, CT, H, W)[b])} for b in range(B)]
    res = run_bass_kernel_spmd(nc, in_maps, core_ids=list(range(N_CORES)))
    o = np.stack([res.results[b]["out"] for b in range(B)])
    return _fold(o)


_NC = None


def _get_nc():
    global _NC
    if _NC is None:
        _NC = build()
    return _NC
